# revision 3
# baseline (speedup 1.0000x reference)
"""Trainium2 Bass kernel for sliding-window GQA attention block.

Reference computation (B=2, S=4096, DIM=1024, H=16 q-heads, KV=2 kv-heads,
D=64, W=256 window):
    q = x@Wq + bq ; k = x@Wk + bk ; v = x@Wv + bv        (GQA repeat kv x8)
    local attention: query t attends keys [t-128, t+128) (zero-padded edges,
    no 1/sqrt(d) scaling), softmax, out = probs@v
    y = out@Wo + bo

Sharding: 8 cores = batch(2) x seq-quarter(4). Each core computes 1024
query rows end-to-end (all 16 heads) from a 1280-row haloed x slice.
No cross-core communication; host pads/transposes/gathers.

On-device pipeline per core (all matmuls bf16, fp32 PSUM accumulation):
  QKV projections -> scores (queries on partitions, row-packed head pairs,
  additive -1e30 band masks via identity-matmuls) -> exp with fused
  per-partition accumulate (softmax denominators) -> PE-transpose of probs
  -> probs@V -> divide by denominator (per-partition scalar) -> transpose
  -> out-projection with K=1 bias-row fold.
"""

import functools
import numpy as np

B, S, DIM = 2, 4096, 1024
H, KV, D = 16, 2, 64
W, HW = 256, 128
NCORES = 8
QT = 4           # sequence quarters
T = S // QT      # 1024 query rows per core
TH = T + 2 * HW  # 1280 haloed rows
NEG = -1e30


@functools.lru_cache(maxsize=1)
def _build_nc():
    import concourse.bacc as bacc
    import concourse.tile as tile
    from concourse import mybir
    from concourse.masks import make_identity

    f32 = mybir.dt.float32
    bf16 = mybir.dt.bfloat16
    Exp = mybir.ActivationFunctionType.Exp
    Identity = mybir.ActivationFunctionType.Identity

    nc = bacc.Bacc("TRN2", target_bir_lowering=False, debug=False)

    xT = nc.dram_tensor("xT", [DIM, TH], bf16, kind="ExternalInput")
    wq = nc.dram_tensor("Wq", [DIM, DIM], bf16, kind="ExternalInput")
    wk = nc.dram_tensor("Wk", [DIM, KV * D], bf16, kind="ExternalInput")
    wv = nc.dram_tensor("Wv", [DIM, KV * D], bf16, kind="ExternalInput")
    wo = nc.dram_tensor("Wo", [DIM, DIM], bf16, kind="ExternalInput")
    bqc = nc.dram_tensor("bqc", [128, 8], f32, kind="ExternalInput")
    bk_row = nc.dram_tensor("bk_row", [1, KV * D], bf16, kind="ExternalInput")
    bv_row = nc.dram_tensor("bv_row", [1, KV * D], bf16, kind="ExternalInput")
    bo_row = nc.dram_tensor("bo_row", [1, DIM], bf16, kind="ExternalInput")
    ind = nc.dram_tensor("ind", [1, TH], bf16, kind="ExternalInput")
    out = nc.dram_tensor("out", [T, DIM], f32, kind="ExternalOutput")

    with tile.TileContext(nc) as tc:
        with tc.tile_pool(name="const", bufs=1) as const, \
             tc.tile_pool(name="w", bufs=1) as wpool, \
             tc.tile_pool(name="act", bufs=1) as actp, \
             tc.tile_pool(name="attn", bufs=2) as attnp, \
             tc.tile_pool(name="ps", bufs=2, space="PSUM") as ps:

            # ---- constants -------------------------------------------------
            ident = const.tile([128, 128], bf16, tag="ident")
            make_identity(nc, ident)
            # band masks: tile column c (u offset), partition p (query).
            # maskP1 (left u-chunk): invalid where c < p.
            maskP1 = const.tile([128, 128], bf16, tag="p1")
            nc.gpsimd.memset(maskP1, 0.0)
            nc.gpsimd.affine_select(
                out=maskP1, in_=maskP1, compare_op=mybir.AluOpType.is_ge,
                fill=NEG, base=0, pattern=[[1, 128]], channel_multiplier=-1)
            # maskP2 (right u-chunk): invalid where c >= p.
            maskP2 = const.tile([128, 128], bf16, tag="p2")
            nc.gpsimd.memset(maskP2, 0.0)
            nc.gpsimd.affine_select(
                out=maskP2, in_=maskP2, compare_op=mybir.AluOpType.is_ge,
                fill=NEG, base=-1, pattern=[[-1, 128]], channel_multiplier=1)
            ones_row = const.tile([1, 128], bf16, tag="ones")
            nc.vector.memset(ones_row, 1.0)

            bq_sb = const.tile([128, 8], f32, tag="bq")
            nc.sync.dma_start(out=bq_sb, in_=bqc[:, :])
            bkr = const.tile([1, KV * D], bf16, tag="bkr")
            nc.sync.dma_start(out=bkr, in_=bk_row[:, :])
            bvr = const.tile([1, KV * D], bf16, tag="bvr")
            nc.sync.dma_start(out=bvr, in_=bv_row[:, :])
            bor = const.tile([1, DIM], bf16, tag="bor")
            nc.sync.dma_start(out=bor, in_=bo_row[:, :])
            ind_sb = const.tile([1, TH], bf16, tag="ind")
            nc.sync.dma_start(out=ind_sb, in_=ind[:, :])

            # ---- weight/activation loads ----------------------------------
            xT_sb = []
            wq_sb, wk_sb, wv_sb, wo_sb = [], [], [], []
            for k in range(8):
                t_x = wpool.tile([128, TH], bf16, tag=f"xT{k}", name=f"xT{k}")
                nc.sync.dma_start(out=t_x, in_=xT[k * 128:(k + 1) * 128, :])
                xT_sb.append(t_x)
                t_q = wpool.tile([128, DIM], bf16, tag=f"wq{k}", name=f"wq{k}")
                nc.sync.dma_start(out=t_q, in_=wq[k * 128:(k + 1) * 128, :])
                wq_sb.append(t_q)
                t_k = wpool.tile([128, KV * D], bf16, tag=f"wk{k}", name=f"wk{k}")
                nc.sync.dma_start(out=t_k, in_=wk[k * 128:(k + 1) * 128, :])
                wk_sb.append(t_k)
                t_v = wpool.tile([128, KV * D], bf16, tag=f"wv{k}", name=f"wv{k}")
                nc.sync.dma_start(out=t_v, in_=wv[k * 128:(k + 1) * 128, :])
                wv_sb.append(t_v)
                t_o = wpool.tile([128, DIM], bf16, tag=f"wo{k}", name=f"wo{k}")
                nc.sync.dma_start(out=t_o, in_=wo[k * 128:(k + 1) * 128, :])
                wo_sb.append(t_o)

            # ---- Q projection: qT[m] holds heads (m, m+8) on partition
            # halves (row-packed pairs for the scores matmuls) --------------
            qT_sb = []
            for m in range(8):
                t_qt = actp.tile([128, T], bf16, tag=f"qT{m}", name=f"qT{m}")
                qT_sb.append(t_qt)
            for m in range(8):
                for n in range(2):
                    q_ps = ps.tile([128, 512], f32, tag="proj", name="q_ps")
                    for k in range(8):
                        nc.tensor.matmul(
                            out=q_ps,
                            lhsT=wq_sb[k][:, m * 128:(m + 1) * 128],
                            rhs=xT_sb[k][:, HW + n * 512: HW + (n + 1) * 512],
                            start=(k == 0), stop=(k == 7))
                    nc.scalar.activation(
                        out=qT_sb[m][:, n * 512:(n + 1) * 512], in_=q_ps,
                        func=Identity, bias=bq_sb[:, m:m + 1], scale=1.0)

            # ---- K projection over halo; zero at padded rows via ind fold -
            kT_sb = actp.tile([128, TH], bf16, tag="kT")
            for c0, cw in ((0, 512), (512, 512), (1024, 256)):
                k_ps = ps.tile([128, 512], f32, tag="proj", name="k_ps")
                for k in range(8):
                    nc.tensor.matmul(
                        out=k_ps[:, :cw], lhsT=wk_sb[k],
                        rhs=xT_sb[k][:, c0:c0 + cw],
                        start=(k == 0), stop=False)
                nc.tensor.matmul(
                    out=k_ps[:, :cw], lhsT=bkr, rhs=ind_sb[:, c0:c0 + cw],
                    start=False, stop=True)
                nc.scalar.copy(out=kT_sb[:, c0:c0 + cw], in_=k_ps[:, :cw])

            # ---- V projection (keys on partitions) ------------------------
            v_sb = actp.tile([128, TH], bf16, tag="V")
            for ut in range(TH // 128):
                v_ps = ps.tile([128, 512], f32, tag="proj", name="v_ps")
                for k in range(8):
                    nc.tensor.matmul(
                        out=v_ps[:, :KV * D],
                        lhsT=xT_sb[k][:, ut * 128:(ut + 1) * 128],
                        rhs=wv_sb[k], start=(k == 0), stop=False)
                nc.tensor.matmul(
                    out=v_ps[:, :KV * D],
                    lhsT=ind_sb[:, ut * 128:(ut + 1) * 128], rhs=bvr,
                    start=False, stop=True)
                nc.vector.tensor_copy(
                    out=v_sb[:, ut * 128:(ut + 1) * 128], in_=v_ps[:, :KV * D])

            # ---- attention + output transpose -----------------------------
            attnT = actp.tile([128, 8 * T], bf16, tag="attnT")
            attnT_v = attnT.rearrange("p (k t) -> p k t", k=8)
            for blk in range(4):
                for tt in range(2):
                    qcol = blk * 256 + tt * 128
                    u0 = qcol  # halo col of first attended key
                    denoms = attnp.tile([128, 16], f32, tag="den")
                    recip = attnp.tile([128, 16], f32, tag="rec")
                    p_tiles = []
                    for m in range(8):
                        for half in range(2):
                            h = m + 8 * half
                            s_ps = ps.tile([128, 384], f32, tag="s", name="s_ps")
                            nc.tensor.matmul(
                                out=s_ps,
                                lhsT=qT_sb[m][half * 64:(half + 1) * 64,
                                              qcol:qcol + 128],
                                rhs=kT_sb[half * 64:(half + 1) * 64,
                                          u0:u0 + 384],
                                start=True, stop=False,
                                tile_position=(64 * half, 0))
                            nc.tensor.matmul(
                                out=s_ps[:, 0:128], lhsT=ident, rhs=maskP1,
                                start=False, stop=False)
                            nc.tensor.matmul(
                                out=s_ps[:, 256:384], lhsT=ident, rhs=maskP2,
                                start=False, stop=True)
                            p_t = attnp.tile([128, 384], bf16, tag="P",
                                             bufs=18, name="p_t")
                            nc.scalar.activation(
                                out=p_t, in_=s_ps, func=Exp,
                                accum_out=denoms[:, h:h + 1])
                            p_tiles.append((h, p_t))
                    nc.vector.reciprocal(out=recip, in_=denoms)
                    attn_t = attnp.tile([128, DIM], bf16, tag="attn")
                    for h, p_t in p_tiles:
                        pt_ps = ps.tile([128, 384], bf16, tag="pt", name="pt_ps")
                        for j in range(3):
                            nc.tensor.matmul(
                                out=pt_ps[:, j * 128:(j + 1) * 128],
                                lhsT=p_t[:, j * 128:(j + 1) * 128], rhs=ident,
                                is_transpose=True,
                                start=(j == 0), stop=(j == 2))
                        p_T = attnp.tile([128, 384], bf16, tag="PT", bufs=3,
                                         name="p_T")
                        if h % 2 == 0:
                            nc.scalar.copy(out=p_T, in_=pt_ps)
                        else:
                            nc.vector.tensor_copy(out=p_T, in_=pt_ps)
                        o_ps = ps.tile([128, 64], f32, tag="o", name="o_ps")
                        kv = h // 8
                        for j in range(3):
                            ut = blk * 2 + tt + j
                            nc.tensor.matmul(
                                out=o_ps,
                                lhsT=p_T[:, j * 128:(j + 1) * 128],
                                rhs=v_sb[:, ut * 128 + kv * 64:
                                         ut * 128 + kv * 64 + 64],
                                start=(j == 0), stop=(j == 2))
                        nc.vector.tensor_scalar_mul(
                            attn_t[:, h * 64:(h + 1) * 64], o_ps,
                            recip[:, h:h + 1])
                    # transpose attn rows (t) x cols (hd) -> attnT k-tiles
                    for g in range(3):
                        kcnt = 3 if g < 2 else 2
                        at_ps = ps.tile([128, 384], bf16, tag="pt",
                                        name="at_ps")
                        for jj in range(kcnt):
                            kk = g * 3 + jj
                            nc.tensor.matmul(
                                out=at_ps[:, jj * 128:(jj + 1) * 128],
                                lhsT=attn_t[:, kk * 128:(kk + 1) * 128],
                                rhs=ident, is_transpose=True,
                                start=(jj == 0), stop=(jj == kcnt - 1))
                        src = at_ps[:, :kcnt * 128].rearrange(
                            "p (j c) -> p j c", j=kcnt)
                        dst = attnT_v[:, g * 3:g * 3 + kcnt, qcol:qcol + 128]
                        if tt == 0:
                            nc.scalar.copy(out=dst, in_=src)
                        else:
                            nc.vector.tensor_copy(out=dst, in_=src)

            # ---- output projection + bias row fold ------------------------
            for mt in range(8):
                out_t = attnp.tile([128, DIM], f32, tag="outt")
                for n in range(2):
                    o2 = ps.tile([128, 512], f32, tag="proj", name="o2_ps")
                    for k in range(8):
                        nc.tensor.matmul(
                            out=o2,
                            lhsT=attnT[:, k * T + mt * 128:
                                       k * T + (mt + 1) * 128],
                            rhs=wo_sb[k][:, n * 512:(n + 1) * 512],
                            start=(k == 0), stop=False)
                    nc.tensor.matmul(
                        out=o2, lhsT=ones_row,
                        rhs=bor[:, n * 512:(n + 1) * 512],
                        start=False, stop=True)
                    nc.scalar.copy(out=out_t[:, n * 512:(n + 1) * 512], in_=o2)
                nc.sync.dma_start(out=out[mt * 128:(mt + 1) * 128, :],
                                  in_=out_t)

    nc.compile()
    return nc


def _host_prep(x, Wq, bq, Wk, bk, Wv, bv, Wo, bo):
    import ml_dtypes
    bf16 = ml_dtypes.bfloat16

    # permute Wq/bq columns so qT m-tile holds head m on partitions 0-63 and
    # head m+8 on partitions 64-127 (enables row-packed score matmuls)
    idx = np.empty(DIM, dtype=np.int64)
    for m in range(8):
        for j in range(128):
            h = m if j < 64 else m + 8
            idx[m * 128 + j] = h * D + (j % 64)
    wq_p = np.ascontiguousarray(Wq[:, idx]).astype(bf16)
    bq_p = bq[idx].astype(np.float32).reshape(8, 128).T.copy()  # (128, 8)
    wk_b = np.ascontiguousarray(Wk).astype(bf16)
    wv_b = np.ascontiguousarray(Wv).astype(bf16)
    wo_b = np.ascontiguousarray(Wo).astype(bf16)
    bk_r = bk.reshape(1, KV * D).astype(bf16)
    bv_r = bv.reshape(1, KV * D).astype(bf16)
    bo_r = bo.reshape(1, DIM).astype(bf16)

    in_maps = []
    for c in range(NCORES):
        b, qt = c // QT, c % QT
        lo, hi = qt * T - HW, qt * T + T + HW
        xs = np.zeros((TH, DIM), dtype=np.float32)
        s0, s1 = max(lo, 0), min(hi, S)
        xs[s0 - lo:s1 - lo] = x[b, s0:s1]
        ind_r = np.zeros((1, TH), dtype=bf16)
        ind_r[0, s0 - lo:s1 - lo] = 1.0
        in_maps.append({
            "xT": np.ascontiguousarray(xs.T).astype(bf16),
            "Wq": wq_p, "Wk": wk_b, "Wv": wv_b, "Wo": wo_b,
            "bqc": bq_p, "bk_row": bk_r, "bv_row": bv_r, "bo_row": bo_r,
            "ind": ind_r,
        })
    return in_maps


def kernel(x, Wq, bq, Wk, bk, Wv, bv, Wo, bo):
    from concourse.bass_utils import run_bass_kernel_spmd

    nc = _build_nc()
    in_maps = _host_prep(x, Wq, bq, Wk, bk, Wv, bv, Wo, bo)
    res = run_bass_kernel_spmd(nc, in_maps, core_ids=list(range(NCORES)))
    out = np.empty((B, S, DIM), dtype=np.float32)
    for c in range(NCORES):
        b, qt = c // QT, c % QT
        out[b, qt * T:(qt + 1) * T] = res.results[c]["out"]
    return out


# revision 6
# speedup vs baseline: 1.0266x; 1.0266x over previous
"""Trainium2 Bass kernel for sliding-window GQA attention block.

Reference computation (B=2, S=4096, DIM=1024, H=16 q-heads, KV=2 kv-heads,
D=64, W=256 window):
    q = x@Wq + bq ; k = x@Wk + bk ; v = x@Wv + bv        (GQA repeat kv x8)
    local attention: query t attends keys [t-128, t+128) (zero-padded edges,
    no 1/sqrt(d) scaling), softmax, out = probs@v
    y = out@Wo + bo

Sharding: 8 cores = batch(2) x seq-quarter(4). Each core computes 1024
query rows end-to-end (all 16 heads) from a 1280-row haloed x slice.
No cross-core communication; host pads/transposes/gathers.

On-device pipeline per core (all matmuls bf16, fp32 PSUM accumulation):
  QKV projections -> scores (queries on partitions, row-packed head pairs,
  additive -1e30 band masks via identity-matmuls) -> exp with fused
  per-partition accumulate (softmax denominators) -> PE-transpose of probs
  -> probs@V -> divide by denominator (per-partition scalar) -> transpose
  -> out-projection with K=1 bias-row fold.
"""

import functools
import numpy as np

B, S, DIM = 2, 4096, 1024
H, KV, D = 16, 2, 64
W, HW = 256, 128
NCORES = 8
QT = 4           # sequence quarters
T = S // QT      # 1024 query rows per core
TH = T + 2 * HW  # 1280 haloed rows
NEG = -1e30


@functools.lru_cache(maxsize=1)
def _build_nc():
    import concourse.bacc as bacc
    import concourse.tile as tile
    from concourse import mybir
    from concourse.masks import make_identity

    f32 = mybir.dt.float32
    bf16 = mybir.dt.bfloat16
    Exp = mybir.ActivationFunctionType.Exp
    Identity = mybir.ActivationFunctionType.Identity

    nc = bacc.Bacc("TRN2", target_bir_lowering=False, debug=False)

    xT = nc.dram_tensor("xT", [DIM, TH], bf16, kind="ExternalInput")
    wq = nc.dram_tensor("Wq", [DIM, DIM], bf16, kind="ExternalInput")
    wk = nc.dram_tensor("Wk", [DIM, KV * D], bf16, kind="ExternalInput")
    wv = nc.dram_tensor("Wv", [DIM, KV * D], bf16, kind="ExternalInput")
    wo = nc.dram_tensor("Wo", [DIM, DIM], bf16, kind="ExternalInput")
    bqc = nc.dram_tensor("bqc", [128, 8], f32, kind="ExternalInput")
    bk_row = nc.dram_tensor("bk_row", [1, KV * D], bf16, kind="ExternalInput")
    bv_row = nc.dram_tensor("bv_row", [1, KV * D], bf16, kind="ExternalInput")
    bo_row = nc.dram_tensor("bo_row", [1, DIM], bf16, kind="ExternalInput")
    ind = nc.dram_tensor("ind", [1, TH], bf16, kind="ExternalInput")
    out = nc.dram_tensor("out", [T, DIM], f32, kind="ExternalOutput")

    with tile.TileContext(nc) as tc:
        with tc.tile_pool(name="const", bufs=1) as const, \
             tc.tile_pool(name="w", bufs=1) as wpool, \
             tc.tile_pool(name="act", bufs=1) as actp, \
             tc.tile_pool(name="attn", bufs=2) as attnp, \
             tc.tile_pool(name="ps", bufs=2, space="PSUM") as ps:

            # ---- constants -------------------------------------------------
            ident = const.tile([128, 128], bf16, tag="ident")
            make_identity(nc, ident)
            # 0/1 window mask (query partition p, score column c in [0,384)):
            # chunk0 valid where c >= p, chunk1 all-valid, chunk2 valid where
            # c < p. Applied multiplicatively on exp(scores) by the fused
            # DVE tensor_tensor_reduce that also produces softmax denominators.
            mask01 = const.tile([128, 384], bf16, tag="mask01")
            nc.gpsimd.memset(mask01, 1.0)
            nc.gpsimd.affine_select(
                out=mask01[:, 0:128], in_=mask01[:, 0:128],
                compare_op=mybir.AluOpType.is_ge,
                fill=0.0, base=0, pattern=[[1, 128]], channel_multiplier=-1)
            nc.gpsimd.affine_select(
                out=mask01[:, 256:384], in_=mask01[:, 256:384],
                compare_op=mybir.AluOpType.is_ge,
                fill=0.0, base=-1, pattern=[[-1, 128]], channel_multiplier=1)
            ones_row = const.tile([1, 128], bf16, tag="ones")
            nc.vector.memset(ones_row, 1.0)

            bq_sb = const.tile([128, 8], f32, tag="bq")
            nc.sync.dma_start(out=bq_sb, in_=bqc[:, :])
            bkr = const.tile([1, KV * D], bf16, tag="bkr")
            nc.sync.dma_start(out=bkr, in_=bk_row[:, :])
            bvr = const.tile([1, KV * D], bf16, tag="bvr")
            nc.sync.dma_start(out=bvr, in_=bv_row[:, :])
            bor = const.tile([1, DIM], bf16, tag="bor")
            nc.sync.dma_start(out=bor, in_=bo_row[:, :])
            ind_sb = const.tile([1, TH], bf16, tag="ind")
            nc.sync.dma_start(out=ind_sb, in_=ind[:, :])

            # ---- weight/activation loads ----------------------------------
            xT_sb = []
            wq_sb, wk_sb, wv_sb, wo_sb = [], [], [], []
            for k in range(8):
                t_x = wpool.tile([128, TH], bf16, tag=f"xT{k}", name=f"xT{k}")
                nc.sync.dma_start(out=t_x, in_=xT[k * 128:(k + 1) * 128, :])
                xT_sb.append(t_x)
                t_q = wpool.tile([128, DIM], bf16, tag=f"wq{k}", name=f"wq{k}")
                nc.sync.dma_start(out=t_q, in_=wq[k * 128:(k + 1) * 128, :])
                wq_sb.append(t_q)
                t_k = wpool.tile([128, KV * D], bf16, tag=f"wk{k}", name=f"wk{k}")
                nc.sync.dma_start(out=t_k, in_=wk[k * 128:(k + 1) * 128, :])
                wk_sb.append(t_k)
                t_v = wpool.tile([128, KV * D], bf16, tag=f"wv{k}", name=f"wv{k}")
                nc.sync.dma_start(out=t_v, in_=wv[k * 128:(k + 1) * 128, :])
                wv_sb.append(t_v)
                t_o = wpool.tile([128, DIM], bf16, tag=f"wo{k}", name=f"wo{k}")
                nc.sync.dma_start(out=t_o, in_=wo[k * 128:(k + 1) * 128, :])
                wo_sb.append(t_o)

            # ---- Q projection: qT[m] holds heads (m, m+8) on partition
            # halves (row-packed pairs for the scores matmuls) --------------
            qT_sb = []
            for m in range(8):
                t_qt = actp.tile([128, T], bf16, tag=f"qT{m}", name=f"qT{m}")
                qT_sb.append(t_qt)
            for m in range(8):
                for n in range(2):
                    q_ps = ps.tile([128, 512], f32, tag="proj", name="q_ps")
                    for k in range(8):
                        nc.tensor.matmul(
                            out=q_ps,
                            lhsT=wq_sb[k][:, m * 128:(m + 1) * 128],
                            rhs=xT_sb[k][:, HW + n * 512: HW + (n + 1) * 512],
                            start=(k == 0), stop=(k == 7))
                    nc.scalar.activation(
                        out=qT_sb[m][:, n * 512:(n + 1) * 512], in_=q_ps,
                        func=Identity, bias=bq_sb[:, m:m + 1], scale=1.0)

            # ---- K projection over halo; zero at padded rows via ind fold -
            kT_sb = actp.tile([128, TH], bf16, tag="kT")
            for c0, cw in ((0, 512), (512, 512), (1024, 256)):
                k_ps = ps.tile([128, 512], f32, tag="proj", name="k_ps")
                for k in range(8):
                    nc.tensor.matmul(
                        out=k_ps[:, :cw], lhsT=wk_sb[k],
                        rhs=xT_sb[k][:, c0:c0 + cw],
                        start=(k == 0), stop=False)
                nc.tensor.matmul(
                    out=k_ps[:, :cw], lhsT=bkr, rhs=ind_sb[:, c0:c0 + cw],
                    start=False, stop=True)
                nc.scalar.copy(out=kT_sb[:, c0:c0 + cw], in_=k_ps[:, :cw])

            # ---- V projection (keys on partitions) ------------------------
            v_sb = actp.tile([128, TH], bf16, tag="V")
            for ut in range(TH // 128):
                v_ps = ps.tile([128, 512], f32, tag="proj", name="v_ps")
                for k in range(8):
                    nc.tensor.matmul(
                        out=v_ps[:, :KV * D],
                        lhsT=xT_sb[k][:, ut * 128:(ut + 1) * 128],
                        rhs=wv_sb[k], start=(k == 0), stop=False)
                nc.tensor.matmul(
                    out=v_ps[:, :KV * D],
                    lhsT=ind_sb[:, ut * 128:(ut + 1) * 128], rhs=bvr,
                    start=False, stop=True)
                nc.vector.tensor_copy(
                    out=v_sb[:, ut * 128:(ut + 1) * 128], in_=v_ps[:, :KV * D])

            # ---- attention + output transpose -----------------------------
            attnT = actp.tile([128, 8 * T], bf16, tag="attnT")
            attnT_v = attnT.rearrange("p (k t) -> p k t", k=8)
            for blk in range(4):
                for tt in range(2):
                    qcol = blk * 256 + tt * 128
                    u0 = qcol  # halo col of first attended key
                    denoms = attnp.tile([128, 16], f32, tag="den")
                    recip = attnp.tile([128, 16], f32, tag="rec")
                    p_tiles = []
                    for m in range(8):
                        for half in range(2):
                            h = m + 8 * half
                            s_ps = ps.tile([128, 384], f32, tag="s", name="s_ps")
                            nc.tensor.matmul(
                                out=s_ps,
                                lhsT=qT_sb[m][half * 64:(half + 1) * 64,
                                              qcol:qcol + 128],
                                rhs=kT_sb[half * 64:(half + 1) * 64,
                                          u0:u0 + 384],
                                start=True, stop=True,
                                tile_position=(64 * half, 0))
                            p_raw = attnp.tile([128, 384], bf16, tag="Praw",
                                               bufs=4, name="p_raw")
                            nc.scalar.activation(out=p_raw, in_=s_ps, func=Exp)
                            p_t = attnp.tile([128, 384], bf16, tag="P",
                                             bufs=18, name="p_t")
                            nc.vector.scalar_tensor_tensor(
                                out=p_t, in0=p_raw, scalar=1.0, in1=mask01,
                                op0=mybir.AluOpType.mult,
                                op1=mybir.AluOpType.mult,
                                accum_out=denoms[:, h:h + 1])
                            p_tiles.append((h, p_t))
                    nc.vector.reciprocal(out=recip, in_=denoms)
                    attn_t = attnp.tile([128, DIM], bf16, tag="attn")
                    for h, p_t in p_tiles:
                        pt_ps = ps.tile([128, 384], bf16, tag="pt", name="pt_ps")
                        for j in range(3):
                            nc.tensor.matmul(
                                out=pt_ps[:, j * 128:(j + 1) * 128],
                                lhsT=p_t[:, j * 128:(j + 1) * 128], rhs=ident,
                                is_transpose=True,
                                start=(j == 0), stop=(j == 2))
                        p_T = attnp.tile([128, 384], bf16, tag="PT", bufs=3,
                                         name="p_T")
                        if h % 2 == 0:
                            nc.scalar.copy(out=p_T, in_=pt_ps)
                        else:
                            nc.vector.tensor_copy(out=p_T, in_=pt_ps)
                        o_ps = ps.tile([128, 64], f32, tag="o", name="o_ps")
                        kv = h // 8
                        for j in range(3):
                            ut = blk * 2 + tt + j
                            nc.tensor.matmul(
                                out=o_ps,
                                lhsT=p_T[:, j * 128:(j + 1) * 128],
                                rhs=v_sb[:, ut * 128 + kv * 64:
                                         ut * 128 + kv * 64 + 64],
                                start=(j == 0), stop=(j == 2))
                        nc.vector.tensor_scalar_mul(
                            attn_t[:, h * 64:(h + 1) * 64], o_ps,
                            recip[:, h:h + 1])
                    # transpose attn rows (t) x cols (hd) -> attnT k-tiles
                    for g in range(3):
                        kcnt = 3 if g < 2 else 2
                        at_ps = ps.tile([128, 384], bf16, tag="pt",
                                        name="at_ps")
                        for jj in range(kcnt):
                            kk = g * 3 + jj
                            nc.tensor.matmul(
                                out=at_ps[:, jj * 128:(jj + 1) * 128],
                                lhsT=attn_t[:, kk * 128:(kk + 1) * 128],
                                rhs=ident, is_transpose=True,
                                start=(jj == 0), stop=(jj == kcnt - 1))
                        src = at_ps[:, :kcnt * 128].rearrange(
                            "p (j c) -> p j c", j=kcnt)
                        dst = attnT_v[:, g * 3:g * 3 + kcnt, qcol:qcol + 128]
                        if tt == 0:
                            nc.scalar.copy(out=dst, in_=src)
                        else:
                            nc.vector.tensor_copy(out=dst, in_=src)

                    # ---- output projection for this query tile (keeps the
                    # PE fed with dense matmuls between attention phases) ----
                    mt = blk * 2 + tt
                    out_t = attnp.tile([128, DIM], f32, tag="outt")
                    for n in range(2):
                        o2 = ps.tile([128, 512], f32, tag="proj", name="o2_ps")
                        for k in range(8):
                            nc.tensor.matmul(
                                out=o2,
                                lhsT=attnT[:, k * T + mt * 128:
                                           k * T + (mt + 1) * 128],
                                rhs=wo_sb[k][:, n * 512:(n + 1) * 512],
                                start=(k == 0), stop=False)
                        nc.tensor.matmul(
                            out=o2, lhsT=ones_row,
                            rhs=bor[:, n * 512:(n + 1) * 512],
                            start=False, stop=True)
                        nc.scalar.copy(out=out_t[:, n * 512:(n + 1) * 512],
                                       in_=o2)
                    nc.sync.dma_start(out=out[mt * 128:(mt + 1) * 128, :],
                                      in_=out_t)

    nc.compile()
    return nc


def _host_prep(x, Wq, bq, Wk, bk, Wv, bv, Wo, bo):
    import ml_dtypes
    bf16 = ml_dtypes.bfloat16

    # permute Wq/bq columns so qT m-tile holds head m on partitions 0-63 and
    # head m+8 on partitions 64-127 (enables row-packed score matmuls)
    idx = np.empty(DIM, dtype=np.int64)
    for m in range(8):
        for j in range(128):
            h = m if j < 64 else m + 8
            idx[m * 128 + j] = h * D + (j % 64)
    wq_p = np.ascontiguousarray(Wq[:, idx]).astype(bf16)
    bq_p = bq[idx].astype(np.float32).reshape(8, 128).T.copy()  # (128, 8)
    wk_b = np.ascontiguousarray(Wk).astype(bf16)
    wv_b = np.ascontiguousarray(Wv).astype(bf16)
    wo_b = np.ascontiguousarray(Wo).astype(bf16)
    bk_r = bk.reshape(1, KV * D).astype(bf16)
    bv_r = bv.reshape(1, KV * D).astype(bf16)
    bo_r = bo.reshape(1, DIM).astype(bf16)

    in_maps = []
    for c in range(NCORES):
        b, qt = c // QT, c % QT
        lo, hi = qt * T - HW, qt * T + T + HW
        xs = np.zeros((TH, DIM), dtype=np.float32)
        s0, s1 = max(lo, 0), min(hi, S)
        xs[s0 - lo:s1 - lo] = x[b, s0:s1]
        ind_r = np.zeros((1, TH), dtype=bf16)
        ind_r[0, s0 - lo:s1 - lo] = 1.0
        in_maps.append({
            "xT": np.ascontiguousarray(xs.T).astype(bf16),
            "Wq": wq_p, "Wk": wk_b, "Wv": wv_b, "Wo": wo_b,
            "bqc": bq_p, "bk_row": bk_r, "bv_row": bv_r, "bo_row": bo_r,
            "ind": ind_r,
        })
    return in_maps


def kernel(x, Wq, bq, Wk, bk, Wv, bv, Wo, bo):
    from concourse.bass_utils import run_bass_kernel_spmd

    nc = _build_nc()
    in_maps = _host_prep(x, Wq, bq, Wk, bk, Wv, bv, Wo, bo)
    res = run_bass_kernel_spmd(nc, in_maps, core_ids=list(range(NCORES)))
    out = np.empty((B, S, DIM), dtype=np.float32)
    for c in range(NCORES):
        b, qt = c // QT, c % QT
        out[b, qt * T:(qt + 1) * T] = res.results[c]["out"]
    return out
